# revision 29
# baseline (speedup 1.0000x reference)
"""GCN block (2x GCNConv + BatchNorm + ReLU) on 8 Trainium2 NeuronCores.

Math: per layer, out = D^-1/2 (A+I) D^-1/2 (x W); then BN (training-mode
stats over nodes) + ReLU.  The bias b is mathematically irrelevant (BN
removes any per-feature constant shift), so it is skipped.

Factorization:
  out[d] = dinv[d] * sum_{e: dst(e)=d} (dinv[src(e)] * h[src(e)])
dinv[src] is folded into the gather source g = dinv * (x @ W) (host-folded
into x for layer 1); dinv[dst] is folded into the one-hot selection matrix S
that turns the per-destination-block scatter-add into PE matmuls:
  S_t[p, c] = (dstloc_t[p] == c) * dinv_dst_t[p]
  zT_block += M_t^T @ S_t     (M_t = dma_gather'ed messages [128 edges, 128 feat])
accumulated in PSUM, feature-major (transposed), so BN stats are a free-dim
reduce and BN+ReLU is one scalar-engine activation.

Sharding: destinations split 5000/core over 8 cores; host bins edges by
(dst-block, src-half) per core.  Nodes use a padded id space (5120/core,
40960 total) so every matmul/DMA tile is uniform.  Both xW phases are
node-sharded (each core computes g only for its own 5120 nodes) and an
AllGather of the 1.3 MB per-core result builds the full gather source;
BN stats come from tiny [128,2] AllReduces of per-core sums.
"""

import contextlib

import numpy as np
import ml_dtypes
import jax
import jax.numpy as jnp
from jax.experimental.shard_map import shard_map
from jax.sharding import Mesh, NamedSharding, PartitionSpec

import concourse.bacc as bacc
import concourse.mybir as mybir
import concourse.tile as tile
from concourse import bass2jax, library_config

BF16 = ml_dtypes.bfloat16
N = 40000
D = 128
NC = 8
SH = N // NC            # 5000 real dst nodes per core
NB = (SH + 127) // 128  # 40 dst blocks per core
SHP = NB * 128          # 5120 padded per core
NP = NC * SHP           # 40960 padded node-id space
NT = NC * NB            # 320 node tiles
HALF = 32768            # int16 index limit split point (in padded ids)
EPS = 1e-5
GT = 4                  # tiles of 128 idx per dma_gather call
NQ = 4                  # SWDGE queues


def _pad_id(ids):
    return (ids // SH) * SHP + (ids % SH)


# ---------------------------------------------------------------- host side

def _bin_edges(edge_index):
    """Bin regular edges by (core, dst-block, src-half) with padded source
    ids; returns per-core gather/selection metadata and the uniform tile
    schedule T[b][h].  Self-loops are NOT binned: their contribution
    dinv[d]^2 * h[d] is injected into each block's PSUM chain via a
    transposing matmul of the per-core xW stage (see message_pass inject),
    but they DO count toward the degree."""
    src = edge_index[0]
    dst = edge_index[1]
    deg = (np.bincount(dst, minlength=N) + 1).astype(np.float32)
    dinv = (1.0 / np.sqrt(deg)).astype(np.float32)

    srcp = _pad_id(src)
    core = dst // SH
    blk = (dst % SH) // 128
    half = (srcp >= HALF).astype(np.int64)
    key = (core * NB + blk) * 2 + half
    order = np.argsort(key, kind="stable")
    srcp_s, dst_s = srcp[order], dst[order]
    cnt = np.bincount(key, minlength=NC * NB * 2).reshape(NC, NB, 2)
    starts = np.concatenate([[0], np.cumsum(cnt.reshape(-1))]).astype(np.int64)

    T = np.ceil(cnt / 128.0).astype(np.int64).max(axis=0)  # [NB, 2]
    TT = int(T.sum())
    seg_off = np.zeros((NB, 2), np.int64)
    off = 0
    for b in range(NB):
        for h in range(2):
            seg_off[b, h] = off
            off += int(T[b, h]) * 128
    assert off == TT * 128

    idx_flat = np.zeros((NC, TT * 128), np.int16)
    dstv_flat = np.full((NC, TT * 128), -1.0, np.float32)
    dinvv_flat = np.zeros((NC, TT * 128), np.float32)
    for c in range(NC):
        for b in range(NB):
            for h in range(2):
                k = (c * NB + b) * 2 + h
                s, e = starts[k], starts[k + 1]
                if e == s:
                    continue
                o = seg_off[b, h]
                n = e - s
                idx_flat[c, o:o + n] = (srcp_s[s:e] - h * HALF).astype(np.int16)
                dstv_flat[c, o:o + n] = (dst_s[s:e] - c * SH - b * 128).astype(np.float32)
                dinvv_flat[c, o:o + n] = dinv[dst_s[s:e]]

    idxw = np.tile(idx_flat.reshape(NC, -1, 16).transpose(0, 2, 1), (1, 8, 1))
    dstv = dstv_flat.reshape(NC, TT, 128).transpose(0, 2, 1).copy()
    dinvv = dinvv_flat.reshape(NC, TT, 128).transpose(0, 2, 1).copy()
    return dinv, idxw, dstv, dinvv, T, seg_off, TT


# ------------------------------------------------------------ device program

def _build_program(T, seg_off, TT, reps=1, gt=None, nq=None, scratch=16384,
                   gbufs=None, use_coll=True, ablate=None):
    gt = gt if gt is not None else GT
    nq = nq if nq is not None else NQ
    gbufs = gbufs if gbufs is not None else 4 * nq
    nc = bacc.Bacc("TRN2", target_bir_lowering=False, debug=False,
                   num_devices=NC, num_swdge_queues=nq,
                   dynamic_dma_scratch_size=scratch)
    dt = mybir.dt

    xTs = nc.dram_tensor("xTs", [D, SHP], dt.bfloat16, kind="ExternalInput").ap()
    W1 = nc.dram_tensor("W1", [D, D], dt.bfloat16, kind="ExternalInput").ap()
    W2 = nc.dram_tensor("W2", [D, D], dt.bfloat16, kind="ExternalInput").ap()
    idxw = nc.dram_tensor("idxw", [128, TT * 8], dt.int16, kind="ExternalInput").ap()
    dstv = nc.dram_tensor("dstv", [128, TT], dt.float32, kind="ExternalInput").ap()
    dinvv = nc.dram_tensor("dinvv", [128, TT], dt.float32, kind="ExternalInput").ap()
    dinv_own = nc.dram_tensor("dinv_own", [128, NB], dt.float32, kind="ExternalInput").ap()
    iota_in = nc.dram_tensor("iota_in", [128, 128], dt.bfloat16, kind="ExternalInput").ap()
    ident_in = nc.dram_tensor("ident_in", [128, 128], dt.bfloat16, kind="ExternalInput").ap()
    bn = nc.dram_tensor("bn", [128, 4], dt.float32, kind="ExternalInput").ap()
    outT = nc.dram_tensor("outT", [128, SH], dt.bfloat16, kind="ExternalOutput").ap()

    g1own = nc.dram_tensor("g1own", [SHP, D], dt.bfloat16, kind="Internal").ap()
    g1d = nc.dram_tensor("g1d", [NP, D], dt.bfloat16, kind="Internal",
                         addr_space="Shared").ap()
    g2own = nc.dram_tensor("g2own", [SHP, D], dt.bfloat16, kind="Internal").ap()
    g2d = nc.dram_tensor("g2d", [NP, D], dt.bfloat16, kind="Internal",
                         addr_space="Shared").ap()
    st1_in = nc.dram_tensor("st1_in", [128, 2], dt.float32, kind="Internal").ap()
    st1_out = nc.dram_tensor("st1_out", [128, 2], dt.float32, kind="Internal",
                             addr_space="Shared").ap()
    st2_in = nc.dram_tensor("st2_in", [128, 2], dt.float32, kind="Internal").ap()
    st2_out = nc.dram_tensor("st2_out", [128, 2], dt.float32, kind="Internal",
                             addr_space="Shared").ap()
    groups = [list(range(NC))]

    with tile.TileContext(nc) as tc:
        nc.gpsimd.load_library(library_config.mlp)
        with contextlib.ExitStack() as ctx:
            cpool = ctx.enter_context(tc.tile_pool(name="consts", bufs=1))
            zpool = ctx.enter_context(tc.tile_pool(name="z", bufs=1))
            gpool = ctx.enter_context(tc.tile_pool(name="gath", bufs=gbufs))
            spool = ctx.enter_context(tc.tile_pool(name="sel", bufs=16))
            stpool = ctx.enter_context(tc.tile_pool(name="stage", bufs=3))
            pspool = ctx.enter_context(tc.tile_pool(name="ps", bufs=4, space="PSUM"))
            smallp = ctx.enter_context(tc.tile_pool(name="small", bufs=1))
            scpool = ctx.enter_context(tc.tile_pool(name="scratch", bufs=1))
            selfp = ctx.enter_context(tc.tile_pool(name="selfS", bufs=1))

            # ---- constants / metadata
            w1sb = cpool.tile([D, D], dt.bfloat16)
            nc.sync.dma_start(w1sb[:], W1[:])
            w2sb = cpool.tile([D, D], dt.bfloat16)
            nc.sync.dma_start(w2sb[:], W2[:])
            iota_sb = cpool.tile([128, 128], dt.bfloat16)
            nc.sync.dma_start(iota_sb[:], iota_in[:])
            idx_sb = cpool.tile([128, TT * 8], dt.int16)
            nc.sync.dma_start(idx_sb[:], idxw[:])
            dstv_sb = cpool.tile([128, TT], dt.float32)
            nc.sync.dma_start(dstv_sb[:], dstv[:])
            dinvv_sb = cpool.tile([128, TT], dt.float32)
            nc.sync.dma_start(dinvv_sb[:], dinvv[:])
            dinv_own_sb = cpool.tile([128, NB], dt.float32)
            nc.sync.dma_start(dinv_own_sb[:], dinv_own[:])
            dinv2_sb = cpool.tile([128, NB], dt.float32)
            nc.vector.tensor_tensor(dinv2_sb[:], dinv_own_sb[:], dinv_own_sb[:],
                                    mybir.AluOpType.mult)
            ident_sb = cpool.tile([128, 128], dt.bfloat16)
            nc.sync.dma_start(ident_sb[:], ident_in[:])
            bn_sb = cpool.tile([128, 4], dt.float32)
            nc.sync.dma_start(bn_sb[:], bn[:])
            eps_sb = cpool.tile([128, 1], dt.float32)
            nc.vector.memset(eps_sb[:], EPS)
            xts_sb = cpool.tile([D, SHP], dt.bfloat16)
            nc.sync.dma_start(xts_sb[:], xTs[:])
            # timing-ablation stand-ins (bench only; never used when ablate=None)
            gconst = sconst = None
            if ablate in ("nogather",):
                gconst = cpool.tile([128, gt, D], dt.bfloat16)
                nc.vector.memset(gconst[:], 0.0)
            if ablate in ("nosel",):
                sconst = cpool.tile([128, 128], dt.bfloat16)
                nc.vector.memset(sconst[:], 0.0)

            def xw_phase(src_sb, wsb, gown, stageS, scaled):
                """g = [dinv *] (src @ W) for this core's NB node tiles.
                src_sb: SBUF [128 feat, SHP] bf16 (feature-major, own shard).
                Main eviction on the scalar engine (per-partition dinv fold
                rides the activation scale); a second eviction on the DVE
                builds stageS[n, j] = dinv * g[n, j] whose transpose is the
                self-loop contribution injected into the MP PSUM chains."""
                sS = dinv_own_sb if not scaled else dinv2_sb
                for t0 in range(0, NB, 8):
                    stage = stpool.tile([128, 8, D], dt.bfloat16, tag="xwstage")
                    for k in range(8):
                        t = t0 + k
                        ps = pspool.tile([128, D], dt.float32, tag="psXW")
                        nc.tensor.matmul(ps[:], src_sb[:, t * 128:(t + 1) * 128],
                                         wsb[:], start=True, stop=True)
                        if scaled:
                            nc.scalar.activation(stage[:, k, :], ps[:],
                                                 mybir.ActivationFunctionType.Copy,
                                                 scale=dinv_own_sb[:, t:t + 1])
                        else:
                            nc.scalar.activation(stage[:, k, :], ps[:],
                                                 mybir.ActivationFunctionType.Copy)
                        nc.vector.tensor_scalar_mul(stageS[:, t, :], ps[:],
                                                    sS[:, t:t + 1])
                    out_ap = gown[t0 * 128:(t0 + 8) * 128, :].rearrange(
                        "(a b) d -> b a d", b=128)
                    nc.sync.dma_start(out_ap, stage[:])

            def message_pass(gsrc, zbuf, qctr, stageS, s1p, s2p):
                """gather + one-hot matmul accumulate -> zT [feat, SHP].
                Each block's PSUM chain opens with the self-loop inject
                (stageS[:,b,:]^T via identity matmul) and closes with two
                scalar evictions whose accum_out columns are the per-block
                BN partial sums (z and z^2)."""
                nomm = ablate == "nomm"
                for b in range(NB):
                    ps = None if nomm else pspool.tile([D, 128], dt.float32,
                                                       tag="ps")
                    ntiles = int(T[b, 0] + T[b, 1]) + 1
                    done = 0
                    if not nomm:
                        nc.tensor.matmul(ps[:], stageS[:, b, :], ident_sb[:],
                                         start=True, stop=(ntiles == 1))
                        done = 1
                    for h in range(2):
                        th = int(T[b, h])
                        if th == 0:
                            continue
                        base = gsrc[0:HALF] if h == 0 else gsrc[HALF:NP]
                        o = int(seg_off[b, h])
                        for c0 in range(0, th, gt):
                            gtn = min(gt, th - c0)
                            nidx = gtn * 128
                            if ablate == "nogather":
                                gt_t = gconst
                            else:
                                gt_t = gpool.tile([128, gt, D], dt.bfloat16,
                                                  tag="gath")
                                nc.gpsimd.dma_gather(
                                    gt_t[:, :gtn, :], base,
                                    idx_sb[:, (o + c0 * 128) // 16:
                                           (o + c0 * 128 + nidx) // 16],
                                    nidx, nidx, D, queue_num=qctr[0] % nq)
                                qctr[0] += 1
                            for t in range(gtn):
                                gti = (o // 128) + c0 + t
                                if ablate == "nosel":
                                    S = sconst
                                else:
                                    S = spool.tile([128, 128], dt.bfloat16,
                                                   tag="sel")
                                    nc.vector.tensor_scalar(
                                        S[:], iota_sb[:], dstv_sb[:, gti:gti + 1],
                                        dinvv_sb[:, gti:gti + 1],
                                        mybir.AluOpType.is_equal,
                                        mybir.AluOpType.mult)
                                if not nomm:
                                    nc.tensor.matmul(ps[:], gt_t[:, t, :], S[:],
                                                     start=(done == 0),
                                                     stop=(done == ntiles - 1))
                                done += 1
                    if not nomm:
                        nc.scalar.activation(zbuf[:, b * 128:(b + 1) * 128],
                                             ps[:],
                                             mybir.ActivationFunctionType.Copy,
                                             accum_out=s1p[:, b:b + 1])
                        sqd = scpool.tile([128, 128], dt.bfloat16, tag="sqd")
                        nc.scalar.activation(sqd[:], ps[:],
                                             mybir.ActivationFunctionType.Square,
                                             accum_out=s2p[:, b:b + 1])

            def bn_coeffs(sums, cnt_inv, layer, tag):
                """(scale, shift) [128,1] from [128,2] {sum z, sum z^2}."""
                mean = smallp.tile([128, 1], dt.float32, tag="mean" + tag)
                nc.vector.tensor_scalar_mul(mean[:], sums[:, 0:1], cnt_inv)
                var = smallp.tile([128, 1], dt.float32, tag="var" + tag)
                nc.vector.tensor_scalar_mul(var[:], sums[:, 1:2], cnt_inv)
                m2 = smallp.tile([128, 1], dt.float32, tag="m2" + tag)
                nc.vector.tensor_tensor(m2[:], mean[:], mean[:], mybir.AluOpType.mult)
                nc.vector.tensor_tensor(var[:], var[:], m2[:], mybir.AluOpType.subtract)
                std = smallp.tile([128, 1], dt.float32, tag="std" + tag)
                nc.scalar.activation(std[:], var[:],
                                     mybir.ActivationFunctionType.Sqrt, bias=eps_sb[:])
                rstd = smallp.tile([128, 1], dt.float32, tag="rstd" + tag)
                nc.vector.reciprocal(rstd[:], std[:])
                scale = smallp.tile([128, 1], dt.float32, tag="scale" + tag)
                nc.vector.tensor_tensor(scale[:], bn_sb[:, 2 * layer:2 * layer + 1],
                                        rstd[:], mybir.AluOpType.mult)
                ms = smallp.tile([128, 1], dt.float32, tag="ms" + tag)
                nc.vector.tensor_tensor(ms[:], mean[:], scale[:], mybir.AluOpType.mult)
                shift = smallp.tile([128, 1], dt.float32, tag="shift" + tag)
                nc.vector.tensor_tensor(shift[:], bn_sb[:, 2 * layer + 1:2 * layer + 2],
                                        ms[:], mybir.AluOpType.subtract)
                return scale, shift

            def bn_stats_allreduce(s1p, s2p, stin, stout, tag):
                """Reduce the per-block partial sums (pads are exactly 0 so
                the padded width is safe), tiny [128,2] AllReduce -> global."""
                stl = smallp.tile([128, 2], dt.float32, tag="stl" + tag)
                nc.vector.tensor_reduce(stl[:, 0:1], s1p[:], mybir.AxisListType.X,
                                        mybir.AluOpType.add)
                nc.vector.tensor_reduce(stl[:, 1:2], s2p[:], mybir.AxisListType.X,
                                        mybir.AluOpType.add)
                nc.gpsimd.dma_start(stin[:], stl[:])
                if use_coll:
                    nc.gpsimd.collective_compute(
                        "AllReduce", mybir.AluOpType.add, replica_groups=groups,
                        ins=[stin], outs=[stout])
                else:
                    nc.sync.dma_start(stout[:], stin[:])
                stg = smallp.tile([128, 2], dt.float32, tag="stg" + tag)
                nc.sync.dma_start(stg[:], stout[:])
                return stg

            for rep in range(reps):
                # ---- phase A: g1own = (dinv*x, host-folded) @ W1, own shard;
                # AllGather the 1.3 MB per-core block into the full g1d.
                stageS1 = selfp.tile([128, NB, D], dt.bfloat16, tag="selfS")
                xw_phase(xts_sb, w1sb, g1own, stageS1, scaled=False)
                if use_coll:
                    nc.gpsimd.collective_compute(
                        "AllGather", mybir.AluOpType.bypass, replica_groups=groups,
                        ins=[g1own], outs=[g1d])
                else:
                    nc.sync.dma_start(g1d[0:SHP, :], g1own[:])

                qctr = [0]
                # ---- phase B: layer-1 message passing -> z1T (sharded)
                z1 = zpool.tile([128, SHP], dt.float32, tag="zT")
                s1pa = smallp.tile([128, NB], dt.float32, tag="s1pa")
                s2pa = smallp.tile([128, NB], dt.float32, tag="s2pa")
                if ablate in ("nomp", "nomm"):
                    nc.vector.memset(z1[:], 0.0)
                    nc.vector.memset(s1pa[:], 0.0)
                    nc.vector.memset(s2pa[:], 0.0)
                if ablate != "nomp":
                    message_pass(g1d, z1, qctr, stageS1, s1pa, s2pa)

                # ---- phase C: BN1 stats via tiny AllReduce, BN+ReLU own shard.
                # Pads of z1 are exactly 0 -> h1 pads = relu(shift1) (finite);
                # their layer-2 contribution is killed by dinv_own = 0 and by
                # never being gathered as a source.
                stg1 = bn_stats_allreduce(s1pa, s2pa, st1_in, st1_out, "a")
                scale1, shift1 = bn_coeffs(stg1, 1.0 / N, 0, "a")
                h1 = scpool.tile([128, SHP], dt.bfloat16, tag="h1")
                nc.scalar.activation(h1[:], z1[:],
                                     mybir.ActivationFunctionType.Relu,
                                     bias=shift1[:], scale=scale1[:])

                # ---- phase D: g2own = dinv * (h1 @ W2), own shard; AllGather.
                stageS2 = selfp.tile([128, NB, D], dt.bfloat16, tag="selfS")
                xw_phase(h1, w2sb, g2own, stageS2, scaled=True)
                if use_coll:
                    nc.gpsimd.collective_compute(
                        "AllGather", mybir.AluOpType.bypass, replica_groups=groups,
                        ins=[g2own], outs=[g2d])
                else:
                    nc.sync.dma_start(g2d[0:SHP, :], g2own[:])

                # ---- phase E: layer-2 message passing -> z2T (sharded)
                z2 = zpool.tile([128, SHP], dt.float32, tag="zT")
                s1pb = smallp.tile([128, NB], dt.float32, tag="s1pb")
                s2pb = smallp.tile([128, NB], dt.float32, tag="s2pb")
                if ablate in ("nomp", "nomm"):
                    nc.vector.memset(z2[:], 0.0)
                    nc.vector.memset(s1pb[:], 0.0)
                    nc.vector.memset(s2pb[:], 0.0)
                if ablate != "nomp":
                    message_pass(g2d, z2, qctr, stageS2, s1pb, s2pb)

                # ---- phase F: BN2 (AllReduce stats) + ReLU -> output
                stg2 = bn_stats_allreduce(s1pb, s2pb, st2_in, st2_out, "b")
                scale2, shift2 = bn_coeffs(stg2, 1.0 / N, 1, "b")
                o = scpool.tile([128, SH], dt.bfloat16, tag="outstage")
                nc.scalar.activation(o[:], z2[:, :SH],
                                     mybir.ActivationFunctionType.Relu,
                                     bias=shift2[:], scale=scale2[:])
                nc.sync.dma_start(outT[:], o[:])

    nc.compile()
    return nc


# ------------------------------------------------------------- pjrt runner

class _SpmdRunner:
    def __init__(self, nc, in_maps, n_cores):
        bass2jax.install_neuronx_cc_hook()
        partition_name = nc.partition_id_tensor.name if nc.partition_id_tensor else None
        in_names, out_names, out_avals, zero_outs = [], [], [], []
        for alloc in nc.m.functions[0].allocations:
            if not isinstance(alloc, mybir.MemoryLocationSet):
                continue
            name = alloc.memorylocations[0].name
            if alloc.kind == "ExternalInput":
                if name != partition_name:
                    in_names.append(name)
            elif alloc.kind == "ExternalOutput":
                out_names.append(name)
                shape = tuple(alloc.tensor_shape)
                dtype = mybir.dt.np(alloc.dtype)
                out_avals.append(jax.core.ShapedArray(shape, dtype))
                zero_outs.append(np.zeros(shape, dtype))
        n_params = len(in_names)
        all_in = list(in_names) + out_names + ([partition_name] if partition_name else [])

        def _body(*args):
            operands = list(args)
            if partition_name is not None:
                operands.append(bass2jax.partition_id_tensor())
            outs = bass2jax._bass_exec_p.bind(
                *operands, out_avals=tuple(out_avals), in_names=tuple(all_in),
                out_names=tuple(out_names), lowering_input_output_aliases=(),
                sim_require_finite=True, sim_require_nnan=True, nc=nc)
            return tuple(outs)

        devices = jax.devices()[:n_cores]
        mesh = Mesh(np.asarray(devices), ("core",))
        in_specs = (PartitionSpec("core"),) * (n_params + len(out_names))
        out_specs = (PartitionSpec("core"),) * len(out_names)
        self.fn = jax.jit(shard_map(_body, mesh=mesh, in_specs=in_specs,
                                    out_specs=out_specs, check_rep=False),
                          keep_unused=True)
        # Inputs/zero-outputs are device_put ONCE with the exact sharding the
        # jitted shard_map expects; otherwise every execution re-streams them
        # over the axon tunnel (~8 ms/exec for the replicated inputs).
        shard = NamedSharding(mesh, PartitionSpec("core"))
        per_core = [[np.asarray(m[nm]) for nm in in_names] for m in in_maps]
        self.concat_in = [
            jax.device_put(np.concatenate([pc[i] for pc in per_core], 0), shard)
            for i in range(n_params)]
        self.zeros_in = [
            jax.device_put(np.zeros((n_cores * z.shape[0],) + z.shape[1:], z.dtype),
                           shard) for z in zero_outs]
        self.out_names, self.out_avals, self.n_cores = out_names, out_avals, n_cores

    def run(self):
        outs = self.fn(*self.concat_in, *self.zeros_in)
        jax.block_until_ready(outs)
        return outs

    def results(self, outs):
        return [{nm: np.asarray(outs[i]).reshape(self.n_cores,
                                                 *self.out_avals[i].shape)[c]
                 for i, nm in enumerate(self.out_names)} for c in range(self.n_cores)]


_CACHE = {}


def _prepare(x, edge_index, W1, b1, g1, be1, W2, b2, g2, be2, reps=1, **bkw):
    x = np.asarray(x, np.float32)
    edge_index = np.asarray(edge_index, np.int64)
    dinv, idxw, dstv, dinvv, T, seg_off, TT = _bin_edges(edge_index)
    key = (TT, tuple(T.reshape(-1)), reps, tuple(sorted(bkw.items())))
    if key not in _CACHE:
        _CACHE[key] = _build_program(T, seg_off, TT, reps=reps, **bkw)
    nc = _CACHE[key]

    xs = (x * dinv[:, None]).astype(BF16)
    xTp = np.zeros((D, NP), BF16)
    dinv_pad = np.zeros(NP, np.float32)
    for c in range(NC):
        xTp[:, c * SHP:c * SHP + SH] = xs[c * SH:(c + 1) * SH].T
        dinv_pad[c * SHP:c * SHP + SH] = dinv[c * SH:(c + 1) * SH]
    # [128, NB] per core: column t holds dinv for the 128 nodes of own tile t
    dinv_all = dinv_pad.reshape(NT, 128).T.copy()   # [128, 320]
    iota_np = np.tile(np.arange(128, dtype=np.float32), (128, 1)).astype(BF16)
    bn_np = np.stack([np.asarray(g1, np.float32), np.asarray(be1, np.float32),
                      np.asarray(g2, np.float32), np.asarray(be2, np.float32)],
                     axis=1)
    in_maps = [{"xTs": np.ascontiguousarray(xTp[:, c * SHP:(c + 1) * SHP]),
                "W1": np.asarray(W1, np.float32).astype(BF16),
                "W2": np.asarray(W2, np.float32).astype(BF16),
                "idxw": idxw[c], "dstv": dstv[c], "dinvv": dinvv[c],
                "dinv_own": np.ascontiguousarray(dinv_all[:, c * NB:(c + 1) * NB]),
                "iota_in": iota_np,
                "ident_in": np.eye(128, dtype=np.float32).astype(BF16),
                "bn": bn_np} for c in range(NC)]
    runner = _SpmdRunner(nc, in_maps, NC)

    def assemble(outs):
        res = runner.results(outs)
        return np.ascontiguousarray(
            np.concatenate([res[c]["outT"] for c in range(NC)], axis=1).T,
            np.float32)
    return runner, assemble


def kernel(x, edge_index, W1, b1, g1, be1, W2, b2, g2, be2):
    runner, assemble = _prepare(x, edge_index, W1, b1, g1, be1, W2, b2, g2, be2)
    return assemble(runner.run())



# revision 35
# speedup vs baseline: 6.5829x; 6.5829x over previous
"""GCN block (2x GCNConv + BatchNorm + ReLU) on 8 Trainium2 NeuronCores.

Math: per layer, out = D^-1/2 (A+I) D^-1/2 (x W); then BN (training-mode
stats over nodes) + ReLU.  The bias b is mathematically irrelevant (BN
removes any per-feature constant shift), so it is skipped.

Factorization:
  out[d] = dinv[d] * sum_{e: dst(e)=d} (dinv[src(e)] * h[src(e)])
dinv[src] is folded into the gather source g = dinv * (x @ W) (host-folded
into x for layer 1); dinv[dst] is folded into the one-hot selection matrix S
that turns the per-destination-block scatter-add into PE matmuls:
  S_t[p, c] = (dstloc_t[p] == c) * dinv_dst_t[p]
  zT_block += M_t^T @ S_t     (M_t = dma_gather'ed messages [128 edges, 128 feat])
accumulated in PSUM, feature-major (transposed), so BN stats are a free-dim
reduce and BN+ReLU is one scalar-engine activation.

Sharding: destinations split 5000/core over 8 cores; host bins edges by
(dst-block, src-half) per core.  Nodes use a padded id space (5120/core,
40960 total) so every matmul/DMA tile is uniform.  Both xW phases are
node-sharded (each core computes g only for its own 5120 nodes) and an
AllGather of the 1.3 MB per-core result builds the full gather source;
BN stats come from tiny [128,2] AllReduces of per-core sums.
"""

import contextlib

import numpy as np
import ml_dtypes
import jax
import jax.numpy as jnp
from jax.experimental.shard_map import shard_map
from jax.sharding import Mesh, NamedSharding, PartitionSpec

import concourse.bacc as bacc
import concourse.mybir as mybir
import concourse.tile as tile
from concourse import bass2jax, library_config

BF16 = ml_dtypes.bfloat16
N = 40000
D = 128
NC = 8
SH = N // NC            # 5000 real dst nodes per core
NB = (SH + 127) // 128  # 40 dst blocks per core
SHP = NB * 128          # 5120 padded per core
NP = NC * SHP           # 40960 padded node-id space
NT = NC * NB            # 320 node tiles
HALF = 32768            # int16 index limit split point (in padded ids)
EPS = 1e-5
GT = 4                  # tiles of 128 idx per dma_gather call
NQ = 4                  # SWDGE queues


def _pad_id(ids):
    return (ids // SH) * SHP + (ids % SH)


# ---------------------------------------------------------------- host side

def _bin_edges(edge_index):
    """Bin regular edges by (core, dst-block, src-half) with padded source
    ids; returns per-core gather/selection metadata and the uniform tile
    schedule T[b][h].  Self-loops are NOT binned: their contribution
    dinv[d]^2 * h[d] is injected into each block's PSUM chain via a
    transposing matmul of the per-core xW stage (see message_pass inject),
    but they DO count toward the degree."""
    src = edge_index[0]
    dst = edge_index[1]
    deg = (np.bincount(dst, minlength=N) + 1).astype(np.float32)
    dinv = (1.0 / np.sqrt(deg)).astype(np.float32)

    srcp = _pad_id(src)
    core = dst // SH
    blk = (dst % SH) // 128
    half = (srcp >= HALF).astype(np.int64)
    key = (core * NB + blk) * 2 + half
    order = np.argsort(key, kind="stable")
    srcp_s, dst_s = srcp[order], dst[order]
    cnt = np.bincount(key, minlength=NC * NB * 2).reshape(NC, NB, 2)
    starts = np.concatenate([[0], np.cumsum(cnt.reshape(-1))]).astype(np.int64)

    T = np.ceil(cnt / 128.0).astype(np.int64).max(axis=0)  # [NB, 2]
    TT = int(T.sum())
    seg_off = np.zeros((NB, 2), np.int64)
    off = 0
    for b in range(NB):
        for h in range(2):
            seg_off[b, h] = off
            off += int(T[b, h]) * 128
    assert off == TT * 128

    idx_flat = np.zeros((NC, TT * 128), np.int16)
    dstv_flat = np.full((NC, TT * 128), -1.0, np.float32)
    dinvv_flat = np.zeros((NC, TT * 128), np.float32)
    for c in range(NC):
        for b in range(NB):
            for h in range(2):
                k = (c * NB + b) * 2 + h
                s, e = starts[k], starts[k + 1]
                if e == s:
                    continue
                o = seg_off[b, h]
                n = e - s
                idx_flat[c, o:o + n] = (srcp_s[s:e] - h * HALF).astype(np.int16)
                dstv_flat[c, o:o + n] = (dst_s[s:e] - c * SH - b * 128).astype(np.float32)
                dinvv_flat[c, o:o + n] = dinv[dst_s[s:e]]

    idxw = np.tile(idx_flat.reshape(NC, -1, 16).transpose(0, 2, 1), (1, 8, 1))
    dstv = dstv_flat.reshape(NC, TT, 128).transpose(0, 2, 1).copy()
    dinvv = dinvv_flat.reshape(NC, TT, 128).transpose(0, 2, 1).copy()
    return dinv, idxw, dstv, dinvv, T, seg_off, TT


# ------------------------------------------------------------ device program

def _build_program(T, seg_off, TT, reps=1, gt=None, nq=None, scratch=65536,
                   gbufs=None, use_coll=True, ablate=None, sp=True, sbufs=16):
    gt = gt if gt is not None else GT
    nq = nq if nq is not None else NQ
    abl = set((ablate or "").split(","))
    gbufs = gbufs if gbufs is not None else 4 * nq
    nc = bacc.Bacc("TRN2", target_bir_lowering=False, debug=False,
                   num_devices=NC, num_swdge_queues=nq,
                   dynamic_dma_scratch_size=scratch)
    dt = mybir.dt

    xTs = nc.dram_tensor("xTs", [D, SHP], dt.bfloat16, kind="ExternalInput").ap()
    W1 = nc.dram_tensor("W1", [D, D], dt.bfloat16, kind="ExternalInput").ap()
    W2 = nc.dram_tensor("W2", [D, D], dt.bfloat16, kind="ExternalInput").ap()
    idxw = nc.dram_tensor("idxw", [128, TT * 8], dt.int16, kind="ExternalInput").ap()
    dstv = nc.dram_tensor("dstv", [128, TT], dt.float32, kind="ExternalInput").ap()
    dinvv = nc.dram_tensor("dinvv", [128, TT], dt.float32, kind="ExternalInput").ap()
    dinv_own = nc.dram_tensor("dinv_own", [128, NB], dt.float32, kind="ExternalInput").ap()
    iota_in = nc.dram_tensor("iota_in", [128, 128], dt.bfloat16, kind="ExternalInput").ap()
    ident_in = nc.dram_tensor("ident_in", [128, 128], dt.bfloat16, kind="ExternalInput").ap()
    bn = nc.dram_tensor("bn", [128, 4], dt.float32, kind="ExternalInput").ap()
    outT = nc.dram_tensor("outT", [128, SH], dt.bfloat16, kind="ExternalOutput").ap()

    g1own = nc.dram_tensor("g1own", [SHP, D], dt.bfloat16, kind="Internal").ap()
    g1d = nc.dram_tensor("g1d", [NP, D], dt.bfloat16, kind="Internal",
                         addr_space="Shared").ap()
    g2own = nc.dram_tensor("g2own", [SHP, D], dt.bfloat16, kind="Internal").ap()
    g2d = nc.dram_tensor("g2d", [NP, D], dt.bfloat16, kind="Internal",
                         addr_space="Shared").ap()
    st1_in = nc.dram_tensor("st1_in", [128, 2], dt.float32, kind="Internal").ap()
    st1_out = nc.dram_tensor("st1_out", [128, 2], dt.float32, kind="Internal",
                             addr_space="Shared").ap()
    st2_in = nc.dram_tensor("st2_in", [128, 2], dt.float32, kind="Internal").ap()
    st2_out = nc.dram_tensor("st2_out", [128, 2], dt.float32, kind="Internal",
                             addr_space="Shared").ap()
    groups = [list(range(NC))]

    with tile.TileContext(nc) as tc:
        nc.gpsimd.load_library(library_config.mlp)
        with contextlib.ExitStack() as ctx:
            cpool = ctx.enter_context(tc.tile_pool(name="consts", bufs=1))
            zpool = ctx.enter_context(tc.tile_pool(name="z", bufs=1))
            gpool = ctx.enter_context(tc.tile_pool(name="gath", bufs=gbufs))
            spool = ctx.enter_context(tc.tile_pool(name="sel", bufs=sbufs))
            stpool = ctx.enter_context(tc.tile_pool(name="stage", bufs=3))
            pspool = ctx.enter_context(tc.tile_pool(name="ps", bufs=4, space="PSUM"))
            smallp = ctx.enter_context(tc.tile_pool(name="small", bufs=1))
            scpool = ctx.enter_context(tc.tile_pool(name="scratch", bufs=1))
            selfp = ctx.enter_context(tc.tile_pool(name="selfS", bufs=1))

            # ---- constants / metadata
            w1sb = cpool.tile([D, D], dt.bfloat16)
            nc.sync.dma_start(w1sb[:], W1[:])
            w2sb = cpool.tile([D, D], dt.bfloat16)
            nc.sync.dma_start(w2sb[:], W2[:])
            iota_sb = cpool.tile([128, 128], dt.bfloat16)
            nc.sync.dma_start(iota_sb[:], iota_in[:])
            idx_sb = cpool.tile([128, TT * 8], dt.int16)
            nc.sync.dma_start(idx_sb[:], idxw[:])
            dstv_sb = cpool.tile([128, TT], dt.float32)
            nc.sync.dma_start(dstv_sb[:], dstv[:])
            dinvv_sb = cpool.tile([128, TT], dt.float32)
            nc.sync.dma_start(dinvv_sb[:], dinvv[:])
            dinv_own_sb = cpool.tile([128, NB], dt.float32)
            nc.sync.dma_start(dinv_own_sb[:], dinv_own[:])
            dinv2_sb = cpool.tile([128, NB], dt.float32)
            nc.vector.tensor_tensor(dinv2_sb[:], dinv_own_sb[:], dinv_own_sb[:],
                                    mybir.AluOpType.mult)
            ident_sb = cpool.tile([128, 128], dt.bfloat16)
            nc.sync.dma_start(ident_sb[:], ident_in[:])
            bn_sb = cpool.tile([128, 4], dt.float32)
            nc.sync.dma_start(bn_sb[:], bn[:])
            eps_sb = cpool.tile([128, 1], dt.float32)
            nc.vector.memset(eps_sb[:], EPS)
            xts_sb = cpool.tile([D, SHP], dt.bfloat16)
            nc.sync.dma_start(xts_sb[:], xTs[:])
            # timing-ablation stand-ins (bench only; never used when ablate=None)
            gconst = sconst = None
            if "nogather" in abl:
                gconst = cpool.tile([128, gt, D], dt.bfloat16)
                nc.vector.memset(gconst[:], 0.0)
            if "nosel" in abl:
                sconst = cpool.tile([128, 128], dt.bfloat16)
                nc.vector.memset(sconst[:], 0.0)

            def xw_phase(src_sb, wsb, gown, stageS, scaled):
                """g = [dinv *] (src @ W) for this core's NB node tiles.
                src_sb: SBUF [128 feat, SHP] bf16 (feature-major, own shard).
                Main eviction on the scalar engine (per-partition dinv fold
                rides the activation scale); a second eviction on the DVE
                builds stageS[n, j] = dinv * g[n, j] whose transpose is the
                self-loop contribution injected into the MP PSUM chains."""
                sS = dinv_own_sb if not scaled else dinv2_sb
                for t0 in range(0, NB, 8):
                    stage = stpool.tile([128, 8, D], dt.bfloat16, tag="xwstage")
                    for k in range(8):
                        t = t0 + k
                        ps = pspool.tile([128, D], dt.float32, tag="psXW")
                        nc.tensor.matmul(ps[:], src_sb[:, t * 128:(t + 1) * 128],
                                         wsb[:], start=True, stop=True)
                        if scaled:
                            nc.scalar.activation(stage[:, k, :], ps[:],
                                                 mybir.ActivationFunctionType.Copy,
                                                 scale=dinv_own_sb[:, t:t + 1])
                        else:
                            nc.scalar.activation(stage[:, k, :], ps[:],
                                                 mybir.ActivationFunctionType.Copy)
                        nc.vector.tensor_scalar_mul(stageS[:, t, :], ps[:],
                                                    sS[:, t:t + 1])
                    out_ap = gown[t0 * 128:(t0 + 8) * 128, :].rearrange(
                        "(a b) d -> b a d", b=128)
                    nc.sync.dma_start(out_ap, stage[:])

            def message_pass(gsrc, zbuf, qctr, stageS, s1p, s2p):
                """gather + one-hot matmul accumulate -> zT [feat, SHP].
                Each block's PSUM chain opens with the self-loop inject
                (stageS[:,b,:]^T via identity matmul) and closes with two
                scalar evictions whose accum_out columns are the per-block
                BN partial sums (z and z^2)."""
                nomm = "nomm" in abl
                for b in range(NB):
                    ps = None if nomm else pspool.tile([D, 128], dt.float32,
                                                       tag="ps")
                    ntiles = int(T[b, 0] + T[b, 1]) + 1
                    done = 0
                    if not nomm:
                        nc.tensor.matmul(ps[:], stageS[:, b, :], ident_sb[:],
                                         start=True, stop=(ntiles == 1))
                        done = 1
                    for h in range(2):
                        th = int(T[b, h])
                        if th == 0:
                            continue
                        base = gsrc[0:HALF] if h == 0 else gsrc[HALF:NP]
                        o = int(seg_off[b, h])
                        for c0 in range(0, th, gt):
                            gtn = min(gt, th - c0)
                            nidx = gtn * 128
                            if "nogather" in abl:
                                gt_t = gconst
                            else:
                                gt_t = gpool.tile([128, gt, D], dt.bfloat16,
                                                  tag="gath")
                                nc.gpsimd.dma_gather(
                                    gt_t[:, :gtn, :], base,
                                    idx_sb[:, (o + c0 * 128) // 16:
                                           (o + c0 * 128 + nidx) // 16],
                                    nidx, nidx, D, queue_num=qctr[0] % nq,
                                    single_packet=sp)
                                qctr[0] += 1
                            for t in range(gtn):
                                gti = (o // 128) + c0 + t
                                if "nosel" in abl:
                                    S = sconst
                                else:
                                    S = spool.tile([128, 128], dt.bfloat16,
                                                   tag="sel")
                                    nc.vector.tensor_scalar(
                                        S[:], iota_sb[:], dstv_sb[:, gti:gti + 1],
                                        dinvv_sb[:, gti:gti + 1],
                                        mybir.AluOpType.is_equal,
                                        mybir.AluOpType.mult)
                                if not nomm:
                                    nc.tensor.matmul(ps[:], gt_t[:, t, :], S[:],
                                                     start=(done == 0),
                                                     stop=(done == ntiles - 1))
                                done += 1
                    if not nomm:
                        nc.scalar.activation(zbuf[:, b * 128:(b + 1) * 128],
                                             ps[:],
                                             mybir.ActivationFunctionType.Copy,
                                             accum_out=s1p[:, b:b + 1])
                        sqd = scpool.tile([128, 128], dt.bfloat16, tag="sqd")
                        nc.scalar.activation(sqd[:], ps[:],
                                             mybir.ActivationFunctionType.Square,
                                             accum_out=s2p[:, b:b + 1])

            def bn_coeffs(sums, cnt_inv, layer, tag):
                """(scale, shift) [128,1] from [128,2] {sum z, sum z^2}."""
                mean = smallp.tile([128, 1], dt.float32, tag="mean" + tag)
                nc.vector.tensor_scalar_mul(mean[:], sums[:, 0:1], cnt_inv)
                var = smallp.tile([128, 1], dt.float32, tag="var" + tag)
                nc.vector.tensor_scalar_mul(var[:], sums[:, 1:2], cnt_inv)
                m2 = smallp.tile([128, 1], dt.float32, tag="m2" + tag)
                nc.vector.tensor_tensor(m2[:], mean[:], mean[:], mybir.AluOpType.mult)
                nc.vector.tensor_tensor(var[:], var[:], m2[:], mybir.AluOpType.subtract)
                std = smallp.tile([128, 1], dt.float32, tag="std" + tag)
                nc.scalar.activation(std[:], var[:],
                                     mybir.ActivationFunctionType.Sqrt, bias=eps_sb[:])
                rstd = smallp.tile([128, 1], dt.float32, tag="rstd" + tag)
                nc.vector.reciprocal(rstd[:], std[:])
                scale = smallp.tile([128, 1], dt.float32, tag="scale" + tag)
                nc.vector.tensor_tensor(scale[:], bn_sb[:, 2 * layer:2 * layer + 1],
                                        rstd[:], mybir.AluOpType.mult)
                ms = smallp.tile([128, 1], dt.float32, tag="ms" + tag)
                nc.vector.tensor_tensor(ms[:], mean[:], scale[:], mybir.AluOpType.mult)
                shift = smallp.tile([128, 1], dt.float32, tag="shift" + tag)
                nc.vector.tensor_tensor(shift[:], bn_sb[:, 2 * layer + 1:2 * layer + 2],
                                        ms[:], mybir.AluOpType.subtract)
                return scale, shift

            def bn_stats_allreduce(s1p, s2p, stin, stout, tag):
                """Reduce the per-block partial sums (pads are exactly 0 so
                the padded width is safe), tiny [128,2] AllReduce -> global."""
                stl = smallp.tile([128, 2], dt.float32, tag="stl" + tag)
                nc.vector.tensor_reduce(stl[:, 0:1], s1p[:], mybir.AxisListType.X,
                                        mybir.AluOpType.add)
                nc.vector.tensor_reduce(stl[:, 1:2], s2p[:], mybir.AxisListType.X,
                                        mybir.AluOpType.add)
                nc.gpsimd.dma_start(stin[:], stl[:])
                if use_coll:
                    nc.gpsimd.collective_compute(
                        "AllReduce", mybir.AluOpType.add, replica_groups=groups,
                        ins=[stin], outs=[stout])
                else:
                    nc.sync.dma_start(stout[:], stin[:])
                stg = smallp.tile([128, 2], dt.float32, tag="stg" + tag)
                nc.sync.dma_start(stg[:], stout[:])
                return stg

            for rep in range(reps):
                # ---- phase A: g1own = (dinv*x, host-folded) @ W1, own shard;
                # AllGather the 1.3 MB per-core block into the full g1d.
                stageS1 = selfp.tile([128, NB, D], dt.bfloat16, tag="selfS")
                xw_phase(xts_sb, w1sb, g1own, stageS1, scaled=False)
                if use_coll:
                    nc.gpsimd.collective_compute(
                        "AllGather", mybir.AluOpType.bypass, replica_groups=groups,
                        ins=[g1own], outs=[g1d])
                else:
                    nc.sync.dma_start(g1d[0:SHP, :], g1own[:])

                qctr = [0]
                # ---- phase B: layer-1 message passing -> z1T (sharded)
                z1 = zpool.tile([128, SHP], dt.float32, tag="zT")
                s1pa = smallp.tile([128, NB], dt.float32, tag="s1pa")
                s2pa = smallp.tile([128, NB], dt.float32, tag="s2pa")
                if abl & {"nomp", "nomm"}:
                    nc.vector.memset(z1[:], 0.0)
                    nc.vector.memset(s1pa[:], 0.0)
                    nc.vector.memset(s2pa[:], 0.0)
                if "nomp" not in abl:
                    message_pass(g1d, z1, qctr, stageS1, s1pa, s2pa)

                # ---- phase C: BN1 stats via tiny AllReduce, BN+ReLU own shard.
                # Pads of z1 are exactly 0 -> h1 pads = relu(shift1) (finite);
                # their layer-2 contribution is killed by dinv_own = 0 and by
                # never being gathered as a source.
                stg1 = bn_stats_allreduce(s1pa, s2pa, st1_in, st1_out, "a")
                scale1, shift1 = bn_coeffs(stg1, 1.0 / N, 0, "a")
                h1 = scpool.tile([128, SHP], dt.bfloat16, tag="h1")
                nc.scalar.activation(h1[:], z1[:],
                                     mybir.ActivationFunctionType.Relu,
                                     bias=shift1[:], scale=scale1[:])

                # ---- phase D: g2own = dinv * (h1 @ W2), own shard; AllGather.
                stageS2 = selfp.tile([128, NB, D], dt.bfloat16, tag="selfS")
                xw_phase(h1, w2sb, g2own, stageS2, scaled=True)
                if use_coll:
                    nc.gpsimd.collective_compute(
                        "AllGather", mybir.AluOpType.bypass, replica_groups=groups,
                        ins=[g2own], outs=[g2d])
                else:
                    nc.sync.dma_start(g2d[0:SHP, :], g2own[:])

                # ---- phase E: layer-2 message passing -> z2T (sharded)
                z2 = zpool.tile([128, SHP], dt.float32, tag="zT")
                s1pb = smallp.tile([128, NB], dt.float32, tag="s1pb")
                s2pb = smallp.tile([128, NB], dt.float32, tag="s2pb")
                if abl & {"nomp", "nomm"}:
                    nc.vector.memset(z2[:], 0.0)
                    nc.vector.memset(s1pb[:], 0.0)
                    nc.vector.memset(s2pb[:], 0.0)
                if "nomp" not in abl:
                    message_pass(g2d, z2, qctr, stageS2, s1pb, s2pb)

                # ---- phase F: BN2 (AllReduce stats) + ReLU -> output
                stg2 = bn_stats_allreduce(s1pb, s2pb, st2_in, st2_out, "b")
                scale2, shift2 = bn_coeffs(stg2, 1.0 / N, 1, "b")
                o = scpool.tile([128, SH], dt.bfloat16, tag="outstage")
                nc.scalar.activation(o[:], z2[:, :SH],
                                     mybir.ActivationFunctionType.Relu,
                                     bias=shift2[:], scale=scale2[:])
                nc.sync.dma_start(outT[:], o[:])

    nc.compile()
    return nc


# ------------------------------------------------------------- pjrt runner

class _SpmdRunner:
    def __init__(self, nc, in_maps, n_cores):
        bass2jax.install_neuronx_cc_hook()
        partition_name = nc.partition_id_tensor.name if nc.partition_id_tensor else None
        in_names, out_names, out_avals, zero_outs = [], [], [], []
        for alloc in nc.m.functions[0].allocations:
            if not isinstance(alloc, mybir.MemoryLocationSet):
                continue
            name = alloc.memorylocations[0].name
            if alloc.kind == "ExternalInput":
                if name != partition_name:
                    in_names.append(name)
            elif alloc.kind == "ExternalOutput":
                out_names.append(name)
                shape = tuple(alloc.tensor_shape)
                dtype = mybir.dt.np(alloc.dtype)
                out_avals.append(jax.core.ShapedArray(shape, dtype))
                zero_outs.append(np.zeros(shape, dtype))
        n_params = len(in_names)
        all_in = list(in_names) + out_names + ([partition_name] if partition_name else [])

        def _body(*args):
            operands = list(args)
            if partition_name is not None:
                operands.append(bass2jax.partition_id_tensor())
            outs = bass2jax._bass_exec_p.bind(
                *operands, out_avals=tuple(out_avals), in_names=tuple(all_in),
                out_names=tuple(out_names), lowering_input_output_aliases=(),
                sim_require_finite=True, sim_require_nnan=True, nc=nc)
            return tuple(outs)

        devices = jax.devices()[:n_cores]
        mesh = Mesh(np.asarray(devices), ("core",))
        in_specs = (PartitionSpec("core"),) * (n_params + len(out_names))
        out_specs = (PartitionSpec("core"),) * len(out_names)
        self.fn = jax.jit(shard_map(_body, mesh=mesh, in_specs=in_specs,
                                    out_specs=out_specs, check_rep=False),
                          keep_unused=True)
        # Inputs/zero-outputs are device_put ONCE with the exact sharding the
        # jitted shard_map expects; otherwise every execution re-streams them
        # over the axon tunnel (~8 ms/exec for the replicated inputs).
        shard = NamedSharding(mesh, PartitionSpec("core"))
        per_core = [[np.asarray(m[nm]) for nm in in_names] for m in in_maps]
        self.concat_in = [
            jax.device_put(np.concatenate([pc[i] for pc in per_core], 0), shard)
            for i in range(n_params)]
        self.zeros_in = [
            jax.device_put(np.zeros((n_cores * z.shape[0],) + z.shape[1:], z.dtype),
                           shard) for z in zero_outs]
        self.out_names, self.out_avals, self.n_cores = out_names, out_avals, n_cores

    def run(self):
        outs = self.fn(*self.concat_in, *self.zeros_in)
        jax.block_until_ready(outs)
        return outs

    def results(self, outs):
        return [{nm: np.asarray(outs[i]).reshape(self.n_cores,
                                                 *self.out_avals[i].shape)[c]
                 for i, nm in enumerate(self.out_names)} for c in range(self.n_cores)]


_CACHE = {}


def _prepare(x, edge_index, W1, b1, g1, be1, W2, b2, g2, be2, reps=1, **bkw):
    x = np.asarray(x, np.float32)
    edge_index = np.asarray(edge_index, np.int64)
    dinv, idxw, dstv, dinvv, T, seg_off, TT = _bin_edges(edge_index)
    key = (TT, tuple(T.reshape(-1)), reps, tuple(sorted(bkw.items())))
    if key not in _CACHE:
        _CACHE[key] = _build_program(T, seg_off, TT, reps=reps, **bkw)
    nc = _CACHE[key]

    xs = (x * dinv[:, None]).astype(BF16)
    xTp = np.zeros((D, NP), BF16)
    dinv_pad = np.zeros(NP, np.float32)
    for c in range(NC):
        xTp[:, c * SHP:c * SHP + SH] = xs[c * SH:(c + 1) * SH].T
        dinv_pad[c * SHP:c * SHP + SH] = dinv[c * SH:(c + 1) * SH]
    # [128, NB] per core: column t holds dinv for the 128 nodes of own tile t
    dinv_all = dinv_pad.reshape(NT, 128).T.copy()   # [128, 320]
    iota_np = np.tile(np.arange(128, dtype=np.float32), (128, 1)).astype(BF16)
    bn_np = np.stack([np.asarray(g1, np.float32), np.asarray(be1, np.float32),
                      np.asarray(g2, np.float32), np.asarray(be2, np.float32)],
                     axis=1)
    in_maps = [{"xTs": np.ascontiguousarray(xTp[:, c * SHP:(c + 1) * SHP]),
                "W1": np.asarray(W1, np.float32).astype(BF16),
                "W2": np.asarray(W2, np.float32).astype(BF16),
                "idxw": idxw[c], "dstv": dstv[c], "dinvv": dinvv[c],
                "dinv_own": np.ascontiguousarray(dinv_all[:, c * NB:(c + 1) * NB]),
                "iota_in": iota_np,
                "ident_in": np.eye(128, dtype=np.float32).astype(BF16),
                "bn": bn_np} for c in range(NC)]
    runner = _SpmdRunner(nc, in_maps, NC)

    def assemble(outs):
        res = runner.results(outs)
        return np.ascontiguousarray(
            np.concatenate([res[c]["outT"] for c in range(NC)], axis=1).T,
            np.float32)
    return runner, assemble


def kernel(x, edge_index, W1, b1, g1, be1, W2, b2, g2, be2):
    runner, assemble = _prepare(x, edge_index, W1, b1, g1, be1, W2, b2, g2, be2)
    return assemble(runner.run())

